# revision 10
# baseline (speedup 1.0000x reference)
"""HA_NET Trainium2 Bass kernel (final: 1.21ms vs 3.43ms baseline).

- gi (word-GRU input projections) kept entirely in SBUF (no DRAM spill,
  no per-step DMA), produced two-ended (t=255,0,254,1,...) and pipelined
  under the scan; non-scan weight transposes deferred under the scan.
- Scan steps slimmed: gi and bhh_n pre-accumulated into PSUM via identity
  matmuls (off the critical chain), one merged sigmoid(rz), h' = n*(1-z)
  + z*h with (1-z) and z*h computed off-chain on GpSimd; chunked
  low-priority PSUM-drain copies split across DVE/Act.
- Word states stored for the conv head as fp8 (e4m3); conv uses fp8
  DoubleRow matmuls; conv max-drains alternate DVE-direct and
  Act-copy(bf16)+DVE-reduce. Recurrence stays bf16 (fp8 recurrence
  matmuls measured slower: LDWEIGHTS loses FWL).
- Sentence-level bi-GRU: both directions batched per step (N=2), paired
  gi_s layout (fwd order + reversed copy), split r/z sigmoid banks,
  grouped identity pre-loads, running mean accumulated into PSUM via an
  identity matmul every 2 steps (no state history buffer).
"""

import numpy as np

NCORES = 8
NS_TOT = 256      # total sentences
T_FULL = 256      # words per sentence
E = 300           # embedding
HWD = 256         # word GRU hidden
HS = 256          # sentence GRU hidden
G = 768           # 3 * hidden (gates r,z,n)
NC_F = 100        # conv filters per width
KWS = [1, 2, 3, 4, 5, 6]
PADS = {1: 0, 2: 0, 3: 1, 4: 1, 5: 2, 6: 2}
USE_DR = True     # fp8 DoubleRow for the conv head
F8W = False       # fp8 weights + state for the word GRU recurrence
F8S = False       # fp8 weights + state for the sentence GRU recurrence
CONV_PS_BF16 = False   # accumulate conv PSUM in bf16 (2x faster max-drains)
SW = 16           # sentence-scan warmup steps (chunked scan)
SCHUNK = 32       # sentences per chunk (8 chunks)
NCH = 16          # chains = 8 chunks x 2 directions
SSTEPS = SW + SCHUNK


def build_program(S, T, n_cores):
    import concourse.bass as bass
    import concourse.bacc as bacc
    import concourse.tile as tile
    import concourse.mybir as mybir
    from concourse import masks
    from contextlib import ExitStack

    dt = mybir.dt
    f32, bf16, f8 = dt.float32, dt.bfloat16, dt.float8e4
    AF = mybir.ActivationFunctionType
    Alu = mybir.AluOpType
    AX = mybir.AxisListType

    NS = S * n_cores          # total sentences
    P2 = 2 * T                # conv concat length
    NT = S * T                # word-positions per core
    ECH = [128, 128, E - 256]
    GM = 6                    # gate chunks of 128
    QPF = 528                 # fp8 conv-state row length (>=514, %16==0)
    NB = T // 16              # gi t-blocks (16)
    KDK = sum(KWS)            # 21 (k,dk) pairs
    MP = 112                  # padded filter block (%16 bytes in fp8)

    nc = bacc.Bacc("TRN2", target_bir_lowering=False, debug=False,
                   num_devices=n_cores)

    # ---------------- DRAM I/O ----------------
    x_d = nc.dram_tensor("x_shard", [S, T, E], f32, kind="ExternalInput").ap()
    wih_w = nc.dram_tensor("wih_w", [G, E], f32, kind="ExternalInput").ap()
    whh_w = nc.dram_tensor("whh_w", [G, HWD], f32, kind="ExternalInput").ap()
    bih_w = nc.dram_tensor("bih_w", [G], f32, kind="ExternalInput").ap()
    bhh_w = nc.dram_tensor("bhh_w", [G], f32, kind="ExternalInput").ap()
    wih_s = nc.dram_tensor("wih_s", [G, 6 * NC_F], f32, kind="ExternalInput").ap()
    whh_s = nc.dram_tensor("whh_s", [G, HS], f32, kind="ExternalInput").ap()
    bih_s = nc.dram_tensor("bih_s", [G], f32, kind="ExternalInput").ap()
    bhh_s = nc.dram_tensor("bhh_s", [G], f32, kind="ExternalInput").ap()
    conv_w = {k: nc.dram_tensor(f"conv{k}_w", [NC_F, 1, k, HWD], f32,
                                kind="ExternalInput").ap() for k in KWS}
    conv_b = {k: nc.dram_tensor(f"conv{k}_b", [NC_F], f32,
                                kind="ExternalInput").ap() for k in KWS}
    fc1_w = nc.dram_tensor("fc1_w", [128, HS], f32, kind="ExternalInput").ap()
    fc1_b = nc.dram_tensor("fc1_b", [128], f32, kind="ExternalInput").ap()
    fc2_w = nc.dram_tensor("fc2_w", [32, 128], f32, kind="ExternalInput").ap()
    fc2_b = nc.dram_tensor("fc2_b", [32], f32, kind="ExternalInput").ap()
    fc3_w = nc.dram_tensor("fc3_w", [1, 32], f32, kind="ExternalInput").ap()
    fc3_b = nc.dram_tensor("fc3_b", [1], f32, kind="ExternalInput").ap()
    out_d = nc.dram_tensor("out", [1, 1], f32, kind="ExternalOutput").ap()

    feats_loc = nc.dram_tensor("feats_local", [NC_F, 6, S], bf16,
                               kind="Internal").ap()
    feats_gat = nc.dram_tensor("feats_gathered", [n_cores, NC_F, 6, S], bf16,
                               kind="Internal", addr_space="Shared").ap()

    with tile.TileContext(nc) as tc, ExitStack() as ctx:
        # ---------------- persistent pools ----------------
        const = ctx.enter_context(tc.tile_pool(name="const", bufs=1))

        ident = const.tile([128, 128], f32)
        masks.make_identity(nc, ident[:])
        identB = const.tile([128, 128], bf16)
        nc.vector.tensor_copy(identB[:], ident[:])

        whhT = const.tile([128, 12 * 128], f8 if F8W else bf16)  # [kc*6+m]
        wihT = const.tile([128, 18 * 128], bf16)        # [kc*6+m]
        wihsT = const.tile([128, 36 * 128], bf16)       # [m*6+k] rows :100
        whhsT = const.tile([128, 12 * 128], f8 if F8S else bf16)
        identF8 = const.tile([128, 128], f8)
        convwT = const.tile([128, 2 * KDK * MP], f8)    # [kc, kdk, mp]
        fc1T = const.tile([128, 2 * 128], bf16)
        fc2T = const.tile([128, 32], bf16)
        fc3T = const.tile([32, 1], bf16)

        biases_w = const.tile([128, 6], f32)   # m<4: bih+bhh ; m>=4: bih
        bhh_w_sb = const.tile([128, 6], f32)
        bih_w_sb = const.tile([128, 6], f32)
        biases_s = const.tile([128, 6], f32)
        bhh_s_sb = const.tile([128, 6], f32)
        bih_s_sb = const.tile([128, 6], f32)
        bhh_n_rep = const.tile([128, 2 * S], bf16)      # [mi, s]
        bhh_s_pair = const.tile([128, 2 * NCH], bf16)   # [mi, ch]
        convb_sb = const.tile([NC_F, 6], f32)
        fc1b_sb = const.tile([128, 1], f32)
        fc2b_sb = const.tile([32, 1], f32)
        fc3b_sb = const.tile([1, 1], f32)

        gis_all = const.tile([128, 6 * NS], bf16)       # [m, n] fwd order
        gis_sc = const.tile([128, 6 * SSTEPS * NCH], bf16)  # [m, i, ch]
        hall_f8 = const.tile([128, S * 2 * QPF], f8)    # [s, c, q]
        hf8_v = hall_f8[:].rearrange("p (s c q) -> p s c q", s=S, c=2)

        # word-scan recurrent state: lives in hall_f8 when F8W, else ping-pong
        if not F8W:
            hf = [const.tile([128, 2 * S], bf16, name=f"hf{j}")
                  for j in range(2)]
            hb = [const.tile([128, 2 * S], bf16, name=f"hb{j}")
                  for j in range(2)]
        sdt = f8 if F8S else bf16
        # [j(2), c(2), ch(NCH)] ping-pong
        scur_all = const.tile([128, 2 * 2 * NCH], sdt)
        scur = [scur_all[:, 0:2 * NCH], scur_all[:, 2 * NCH:4 * NCH]]

        # ---------------- P0: weight prep ----------------
        p0_ctx = ExitStack()
        p0 = p0_ctx.enter_context(tc.tile_pool(name="p0stage", bufs=4))
        p0ps = p0_ctx.enter_context(tc.tile_pool(name="p0psum", bufs=4,
                                                 space="PSUM"))
        _alt = [0]

        def transp(dst_ap, src_ap, rr, cc):
            st = p0.tile([128, 320], f32, tag="p0st")
            nc.sync.dma_start(out=st[:rr, :cc], in_=src_ap)
            ps = p0ps.tile([128, 128], f32, tag="p0ps")
            nc.tensor.matmul(ps[:cc, :rr], st[:rr, :cc], ident[:rr, :rr],
                             is_transpose=True)
            if _alt[0] % 2 == 0:
                nc.scalar.copy(dst_ap, ps[:cc, :rr])
            else:
                nc.vector.tensor_copy(dst_ap, ps[:cc, :rr])
            _alt[0] += 1

        whhT_v = whhT[:].rearrange("p (i q) -> p i q", q=128)
        for kc in range(2):
            for m in range(GM):
                transp(whhT_v[:, kc * 6 + m, :],
                       whh_w[m * 128:(m + 1) * 128, kc * 128:(kc + 1) * 128],
                       128, 128)
        wihT_v = wihT[:].rearrange("p (i q) -> p i q", q=128)
        for kc in range(3):
            cs = ECH[kc]
            for m in range(GM):
                transp(wihT_v[:cs, kc * 6 + m, :],
                       wih_w[m * 128:(m + 1) * 128, kc * 128:kc * 128 + cs],
                       128, cs)
        # non-scan weights are transposed lazily, interleaved with the scan
        DEFERRED = []
        wihsT_v = wihsT[:].rearrange("p (i q) -> p i q", q=128)
        for m in range(GM):
            for k in range(6):
                DEFERRED.append((wihsT_v[:NC_F, m * 6 + k, :],
                                 wih_s[m * 128:(m + 1) * 128,
                                       k * NC_F:(k + 1) * NC_F], 128, NC_F))
        whhsT_v = whhsT[:].rearrange("p (i q) -> p i q", q=128)
        for kc in range(2):
            for m in range(GM):
                DEFERRED.append((whhsT_v[:, kc * 6 + m, :],
                                 whh_s[m * 128:(m + 1) * 128,
                                       kc * 128:(kc + 1) * 128], 128, 128))
        convwT_v = convwT[:].rearrange("p (c i q) -> p c i q", c=2, q=MP)
        conv_idx = {}
        ci = 0
        for k in KWS:
            for dk in range(k):
                conv_idx[(k, dk)] = ci
                for kc in range(2):
                    DEFERRED.append((convwT_v[:, kc, ci, 0:NC_F],
                                     conv_w[k][:, 0, dk,
                                               kc * 128:(kc + 1) * 128],
                                     NC_F, 128))
                ci += 1
        fc1T_v = fc1T[:].rearrange("p (i q) -> p i q", q=128)
        for kc in range(2):
            DEFERRED.append((fc1T_v[:, kc, :],
                             fc1_w[:, kc * 128:(kc + 1) * 128], 128, 128))
        DEFERRED.append((fc2T[:, :], fc2_w[:, :], 32, 128))
        DEFERRED.append((fc3T[:, :], fc3_w[:, :], 1, 32))

        nc.sync.dma_start(out=bih_w_sb[:], in_=bih_w.rearrange("(m p) -> p m", p=128))
        nc.sync.dma_start(out=bhh_w_sb[:], in_=bhh_w.rearrange("(m p) -> p m", p=128))
        nc.sync.dma_start(out=bih_s_sb[:], in_=bih_s.rearrange("(m p) -> p m", p=128))
        nc.sync.dma_start(out=bhh_s_sb[:], in_=bhh_s.rearrange("(m p) -> p m", p=128))
        nc.vector.tensor_add(biases_w[:, 0:4], bih_w_sb[:, 0:4], bhh_w_sb[:, 0:4])
        nc.vector.tensor_copy(biases_w[:, 4:6], bih_w_sb[:, 4:6])
        nc.vector.tensor_add(biases_s[:, 0:4], bih_s_sb[:, 0:4], bhh_s_sb[:, 0:4])
        nc.vector.tensor_copy(biases_s[:, 4:6], bih_s_sb[:, 4:6])
        rep_v = bhh_n_rep[:].rearrange("p (m s) -> p m s", m=2)
        for mi in range(2):
            nc.vector.tensor_scalar(rep_v[:, mi, :], ident[:, 0:S], 0.0,
                                    bhh_w_sb[:, 4 + mi:5 + mi],
                                    Alu.mult, Alu.add)
        pair_v = bhh_s_pair[:].rearrange("p (m ch) -> p m ch", m=2)
        for mi in range(2):
            nc.vector.tensor_scalar(pair_v[:, mi, :], ident[:, 0:NCH], 0.0,
                                    bhh_s_sb[:, 4 + mi:5 + mi],
                                    Alu.mult, Alu.add)
        for j, k in enumerate(KWS):
            nc.sync.dma_start(out=convb_sb[:, j:j + 1], in_=conv_b[k][:, None])
        nc.sync.dma_start(out=fc1b_sb[:], in_=fc1_b[:, None])
        nc.sync.dma_start(out=fc2b_sb[:], in_=fc2_b[:, None])
        nc.sync.dma_start(out=fc3b_sb[:], in_=fc3_b[:, None])
        # zero init states + conv-state guards
        nc.gpsimd.memset(hall_f8[:], 0.0)
        if not F8W:
            nc.gpsimd.memset(hf[0][:], 0.0)
            nc.gpsimd.memset(hb[0][:], 0.0)
        nc.gpsimd.memset(scur[0], 0.0)
        if F8S:
            nc.vector.tensor_copy(identF8[:], ident[:])
        p0_ctx.close()

        # ---------------- P1+P2+P3: gi production pipelined with scan ------
        p123 = ExitStack()
        gibp = p123.enter_context(tc.tile_pool(name="gib", bufs=1))
        p1s = p123.enter_context(tc.tile_pool(name="p1stage", bufs=6))
        gip = p123.enter_context(tc.tile_pool(name="gipsum", bufs=3,
                                              space="PSUM"))
        dwps = p123.enter_context(tc.tile_pool(name="dwpsum", bufs=1,
                                               space="PSUM"))
        xtp = p123.enter_context(tc.tile_pool(name="xT", bufs=3))

        _dw = [0]

        def emit_deferred(n):
            """Emit up to n deferred weight transposes (fills scan idle)."""
            while _dw[0] < len(DEFERRED) and n > 0:
                dst_ap, src_ap, rr, cc = DEFERRED[_dw[0]]
                _dw[0] += 1
                n -= 1
                st = p1s.tile([128, 320], f32, tag="dwst")
                nc.sync.dma_start(out=st[:rr, :cc], in_=src_ap)
                ps = dwps.tile([128, 128], f32, tag="dwps")
                nc.tensor.matmul(ps[:cc, :rr], st[:rr, :cc], ident[:rr, :rr],
                                 is_transpose=True)
                nc.vector.tensor_copy(dst_ap, ps[:cc, :rr])
        scanps = p123.enter_context(tc.tile_pool(name="scanps", bufs=1,
                                                 space="PSUM"))
        scansb = p123.enter_context(tc.tile_pool(name="scansb", bufs=2))

        gib = [gibp.tile([128, 16 * 6 * S], bf16, name=f"gib{b}")
               for b in range(NB)]
        gib_v = [g[:].rearrange("p (t m s) -> p t m s", t=16, m=6)
                 for g in gib]

        def emit_block(b):
            """x load + transpose + gi matmuls for t-block b (16 t's)."""
            xTb = xtp.tile([128, 3 * 512], bf16, tag="xTb")
            xTb_v = xTb[:].rearrange("p (k q) -> p k q", k=3)
            sts = []
            for sub in range(4):
                st = p1s.tile([128, 304], f32, tag="xst")
                t0 = b * 16 + sub * 4
                nc.sync.dma_start(
                    out=st[:, :E],
                    in_=x_d[:, t0:t0 + 4, :].rearrange("s t c -> t s c"))
                sts.append(st)
            for kc in range(3):
                cs = ECH[kc]
                ps = gip.tile([128, 512], f32, tag="gips")
                for sub in range(4):
                    nc.tensor.matmul(ps[:cs, sub * 128:(sub + 1) * 128],
                                     sts[sub][:, kc * 128:kc * 128 + cs],
                                     ident[:], is_transpose=True,
                                     start=(sub == 0), stop=(sub == 3),
                                     skip_group_check=True)
                for hh in range(2):
                    nc.vector.tensor_copy(
                        xTb_v[:cs, kc, hh * 256:(hh + 1) * 256],
                        ps[:cs, hh * 256:(hh + 1) * 256])
            for m in range(GM):
                ps = gip.tile([128, 512], f32, tag="gips")
                for kc in range(3):
                    cs = ECH[kc]
                    nc.tensor.matmul(ps[:], wihT_v[:cs, kc * 6 + m, :],
                                     xTb_v[:cs, kc, :],
                                     start=(kc == 0), stop=(kc == 2))
                psv = ps[:].rearrange("p (t s) -> p t s", t=16)
                for hh in range(2):
                    dst = gib_v[b][:, hh * 8:(hh + 1) * 8, m, :]
                    srcv = psv[:, hh * 8:(hh + 1) * 8, :]
                    if hh == 0:
                        nc.vector.tensor_scalar_add(dst, srcv,
                                                    biases_w[:, m:m + 1])
                    else:
                        nc.scalar.activation(dst, srcv, AF.Identity,
                                             bias=biases_w[:, m:m + 1])

        def word_step(d, i):
            t = i if d == 0 else T - 1 - i
            b, tt = divmod(t, 16)
            pos = t + 1 if d == 0 else t + T + 1
            if F8W:
                pos_prev = t if d == 0 else t + T + 2
                hpc = [hf8_v[:, :, kc, pos_prev] for kc in range(2)]  # [p,s]
                hp_cs = hf8_v[:, :, :, pos_prev].transpose([0, 2, 1])  # [p,c,s]
            else:
                cur = hf if d == 0 else hb
                hprev = cur[i % 2]
                hnext = cur[(i + 1) % 2]
                hprev_v = hprev[:].rearrange("p (c s) -> p c s", c=2)
                hnext_v = hnext[:].rearrange("p (c s) -> p c s", c=2)
                hpc = [hprev_v[:, kc, :] for kc in range(2)]
                hp_cs = hprev_v[:, :, :]

            rz_ps = scanps.tile([128, 4 * S], f32, tag=f"rz{d}")
            n_ps = scanps.tile([128, 2 * S], f32, tag=f"n{d}")
            rzv = rz_ps[:].rearrange("p (m s) -> p m s", m=4)
            nv = n_ps[:].rearrange("p (m s) -> p m s", m=2)

            # pre-accumulate gi (rz) and bhh_n into PSUM (off critical path)
            nc.tensor.matmul(rzv[:, :, :], identB[:],
                             gib_v[b][:, tt, 0:4, :], start=True, stop=False)
            nc.tensor.matmul(nv[:, :, :], identB[:],
                             rep_v[:, :, :], start=True, stop=False)
            for m in range(4):
                for kc in range(2):
                    nc.tensor.matmul(rzv[:, m, :], whhT_v[:, kc * 6 + m, :],
                                     hpc[kc], start=False,
                                     stop=(m == 3 and kc == 1))
            for mi in range(2):
                for kc in range(2):
                    nc.tensor.matmul(nv[:, mi, :], whhT_v[:, kc * 6 + 4 + mi, :],
                                     hpc[kc], start=False,
                                     stop=(mi == 1 and kc == 1))

            rz_sb = scansb.tile([128, 4 * S], bf16, tag=f"rzs{d}")
            rzs = rz_sb[:].rearrange("p (m s) -> p m s", m=4)
            nc.scalar.activation(rzs[:, :, :], rzv[:, :, :], AF.Sigmoid)

            nmix = scansb.tile([128, 2 * S], bf16, tag=f"nm{d}")
            nmv = nmix[:].rearrange("p (m s) -> p m s", m=2)
            nc.vector.tensor_mul(nmv[:, :, :], nv[:, :, :], rzs[:, 0:2, :])
            npre = scansb.tile([128, 2 * S], bf16, tag=f"np{d}")
            npv = npre[:].rearrange("p (m s) -> p m s", m=2)
            nc.vector.tensor_add(npv[:, :, :], nmv[:, :, :],
                                 gib_v[b][:, tt, 4:6, :])

            n_sb = scansb.tile([128, 2 * S], bf16, tag=f"ns{d}")
            nsv = n_sb[:].rearrange("p (m s) -> p m s", m=2)
            nc.scalar.activation(nsv[:, :, :], npv[:, :, :], AF.Tanh)

            oz_sb = scansb.tile([128, 2 * S], bf16, tag=f"oz{d}")
            ozv = oz_sb[:].rearrange("p (m s) -> p m s", m=2)
            nc.gpsimd.tensor_scalar(ozv[:, :, :], rzs[:, 2:4, :], -1.0, 1.0,
                                    Alu.mult, Alu.add)
            zh_sb = scansb.tile([128, 2 * S], bf16, tag=f"zh{d}")
            zhv = zh_sb[:].rearrange("p (m s) -> p m s", m=2)
            nc.gpsimd.tensor_mul(zhv[:, :, :], rzs[:, 2:4, :], hp_cs)

            a_sb = scansb.tile([128, 2 * S], bf16, tag=f"a{d}")
            av = a_sb[:].rearrange("p (m s) -> p m s", m=2)
            nc.vector.tensor_mul(av[:, :, :], nsv[:, :, :], ozv[:, :, :])
            if F8W:
                nc.vector.tensor_add(hf8_v[:, :, :, pos],
                                     av[:, :, :].transpose([0, 2, 1]),
                                     zhv[:, :, :].transpose([0, 2, 1]))
            else:
                nc.vector.tensor_add(hnext_v[:, :, :], av[:, :, :],
                                     zhv[:, :, :])
                nc.vector.tensor_copy(hf8_v[:, :, :, pos],
                                      hnext_v[:, :, :].transpose([0, 2, 1]))

        # interleaved emission: gi blocks two-ended, scan windows behind
        emitted = set()

        def need_block(b):
            if 0 <= b < NB and b not in emitted:
                emitted.add(b)
                emit_block(b)

        need_block(NB - 1)
        need_block(0)
        need_block(NB - 2)
        need_block(1)
        for j in range(NB // 2):
            for i in range(16 * j, 16 * j + 16):
                word_step(0, i)
                word_step(1, i)
                emit_deferred(1)
            need_block(NB - 3 - j)
            need_block(j + 2)
        emit_deferred(len(DEFERRED))
        p123.close()

        # ---------------- P4: conv head (fp8, DoubleRow) ----------------
        maxsb = const.tile([NC_F, 6 * S], f32)
        mxv = maxsb[:].rearrange("p (k s) -> p k s", k=6)
        featsT = const.tile([NC_F, 6 * S], bf16)
        ftv = featsT[:].rearrange("p (k s) -> p k s", k=6)
        with tc.tile_pool(name="convps", bufs=4, space="PSUM") as convps, \
             tc.tile_pool(name="convsb", bufs=2) as convsb:
            for ki, k in enumerate(KWS):
                pad = PADS[k]
                T_out = P2 - k + 1 + 2 * pad
                dks = [pad] + [d for d in range(k) if d != pad]
                for s in range(S):
                    pc = convps.tile([128, 512],
                                     bf16 if CONV_PS_BF16 else f32, tag="cps")
                    for di, dk in enumerate(dks):
                        dlt = dk - pad
                        t0 = max(0, -dlt)
                        t1 = min(T_out, P2 - dlt)
                        st = (di == 0)
                        sp = (di == len(dks) - 1)
                        idx = conv_idx[(k, dk)]
                        if USE_DR:
                            nc.tensor.matmul(
                                pc[:NC_F, t0:t1],
                                convwT_v[:, :, idx, 0:NC_F],
                                hf8_v[:, s, :, 1 + t0 + dlt:1 + t1 + dlt],
                                start=st, stop=sp,
                                perf_mode=mybir.MatmulPerfMode.DoubleRow)
                        else:
                            for kc in range(2):
                                nc.tensor.matmul(
                                    pc[:NC_F, t0:t1],
                                    convwT_v[:, kc, idx, 0:NC_F],
                                    hf8_v[:, s, kc, 1 + t0 + dlt:1 + t1 + dlt],
                                    start=(st and kc == 0),
                                    stop=(sp and kc == 1))
                    if s % 2 == 0:
                        nc.vector.tensor_reduce(
                            mxv[:NC_F, ki, s:s + 1], pc[:NC_F, 0:T_out],
                            axis=AX.X, op=Alu.max)
                    else:
                        scr = convsb.tile([128, 512], bf16, tag="cscr")
                        nc.scalar.copy(scr[:NC_F, :T_out], pc[:NC_F, 0:T_out])
                        nc.vector.tensor_reduce(
                            mxv[:NC_F, ki, s:s + 1], scr[:NC_F, :T_out],
                            axis=AX.X, op=Alu.max)
                nc.scalar.activation(ftv[:NC_F, ki, :], mxv[:NC_F, ki, :],
                                     AF.Sigmoid, bias=convb_sb[:NC_F, ki:ki + 1])
                nc.sync.dma_start(out=feats_loc[:, ki, :],
                                  in_=ftv[:NC_F, ki, :])

        # ---------------- P5: AllGather feats ----------------
        nc.gpsimd.collective_compute(
            "AllGather", Alu.bypass,
            replica_groups=[list(range(n_cores))],
            ins=[feats_loc[:, :, :]],
            outs=[feats_gat[:, :, :, :]])
        featsk = const.tile([NC_F, 6 * NS], bf16)
        fkv = featsk[:].rearrange("p (k n) -> p k n", k=6)
        nc.sync.dma_start(
            out=fkv[:NC_F, :, :],
            in_=feats_gat.rearrange("co o k s -> o k co s"))

        # ---------------- P6: gi_s (fwd order) + chunked-chain staging ----
        gis_av = gis_all[:].rearrange("p (m n) -> p m n", m=6)
        gis_cv = gis_sc[:].rearrange("p (m i ch) -> p m i ch", m=6, i=SSTEPS)
        with tc.tile_pool(name="gisps", bufs=3, space="PSUM") as gisps:
            for m in range(GM):
                ps = gisps.tile([128, NS], f32, tag="gisps")
                for k in range(6):
                    nc.tensor.matmul(ps[:], wihsT_v[:NC_F, m * 6 + k, :],
                                     fkv[:NC_F, k, :],
                                     start=(k == 0), stop=(k == 5))
                nc.scalar.activation(gis_av[:, m, :], ps[:], AF.Identity,
                                     bias=biases_s[:, m:m + 1])
        # stage per-chain gi streams: chain ch = c (fwd) / NCK+c (bwd)
        NCK = NCH // 2
        for c in range(NCK):
            lo = c * SCHUNK - SW
            if lo >= 0:
                nc.vector.tensor_copy(gis_cv[:, :, :, c],
                                      gis_av[:, :, lo:lo + SSTEPS])
            else:
                nc.vector.tensor_copy(gis_cv[:, :, SW:, c],
                                      gis_av[:, :, 0:SCHUNK])
            hi = c * SCHUNK + SCHUNK - 1 + SW  # highest bwd-chain sentence
            if hi <= NS - 1:
                stop = hi - SSTEPS
                nc.vector.tensor_copy(
                    gis_cv[:, :, :, NCK + c],
                    gis_av[:, :, hi:(None if stop < 0 else stop):-1])
            else:
                nc.vector.tensor_copy(
                    gis_cv[:, :, SW:, NCK + c],
                    gis_av[:, :, NS - 1:NS - 1 - SCHUNK:-1])
        # warmup pads of the edge chains: z-gate gi := +30 => z=1, h stays 0
        for ch in (0, NCH - 1):
            for m in range(6):
                nc.vector.tensor_scalar(gis_cv[:, m, 0:SW, ch],
                                        ident[:, 0:SW], 0.0,
                                        30.0 if m in (2, 3) else 0.0,
                                        Alu.mult, Alu.add)

        # ---------------- P7: sentence bi-GRU (chunked, NCH chains) ------
        with tc.tile_pool(name="sps", bufs=1, space="PSUM") as sps, \
             tc.tile_pool(name="ssum_p", bufs=1, space="PSUM") as ssump, \
             tc.tile_pool(name="ssb", bufs=2) as ssb:
            # [j, c, ch] kept-state sums
            ssum = ssump.tile([128, 4 * NCH], f32)
            sident = identF8 if F8S else identB

            for i in range(SSTEPS):
                sprev = scur[i % 2]
                snext = scur[(i + 1) % 2]
                spv = sprev[:].rearrange("p (c ch) -> p c ch", c=2)
                snv = snext[:].rearrange("p (c ch) -> p c ch", c=2)

                sr = sps.tile([128, 2 * NCH], f32, tag="sr")
                sz = sps.tile([128, 2 * NCH], f32, tag="sz")
                sn = sps.tile([128, 2 * NCH], f32, tag="sn")
                srv = sr[:].rearrange("p (m ch) -> p m ch", m=2)
                szv = sz[:].rearrange("p (m ch) -> p m ch", m=2)
                snv_ps = sn[:].rearrange("p (m ch) -> p m ch", m=2)

                # identity pre-loads grouped (one LDWEIGHTS), r-gate MMs first
                nc.tensor.matmul(srv[:, :, :], identB[:],
                                 gis_cv[:, 0:2, i, :], start=True, stop=False)
                nc.tensor.matmul(snv_ps[:, :, :], identB[:],
                                 pair_v[:, :, :], start=True, stop=False)
                nc.tensor.matmul(szv[:, :, :], identB[:],
                                 gis_cv[:, 2:4, i, :], start=True, stop=False)
                for m in range(2):
                    for kc in range(2):
                        nc.tensor.matmul(srv[:, m, :],
                                         whhsT_v[:, kc * 6 + m, :],
                                         spv[:, kc, :], start=False,
                                         stop=(m == 1 and kc == 1))
                for mi in range(2):
                    for kc in range(2):
                        nc.tensor.matmul(snv_ps[:, mi, :],
                                         whhsT_v[:, kc * 6 + 4 + mi, :],
                                         spv[:, kc, :], start=False,
                                         stop=(mi == 1 and kc == 1))
                for m in range(2):
                    for kc in range(2):
                        nc.tensor.matmul(szv[:, m, :],
                                         whhsT_v[:, kc * 6 + 2 + m, :],
                                         spv[:, kc, :], start=False,
                                         stop=(m == 1 and kc == 1))

                sr_sb = ssb.tile([128, 2 * NCH], bf16, tag="srs")
                srs = sr_sb[:].rearrange("p (m ch) -> p m ch", m=2)
                nc.scalar.activation(srs[:, :, :], srv[:, :, :], AF.Sigmoid)

                snmix = ssb.tile([128, 2 * NCH], bf16, tag="snm")
                smv = snmix[:].rearrange("p (m ch) -> p m ch", m=2)
                nc.vector.tensor_mul(smv[:, :, :], snv_ps[:, :, :],
                                     srs[:, :, :])
                snpre = ssb.tile([128, 2 * NCH], bf16, tag="snp")
                spv2 = snpre[:].rearrange("p (m ch) -> p m ch", m=2)
                nc.vector.tensor_add(spv2[:, :, :], smv[:, :, :],
                                     gis_cv[:, 4:6, i, :])

                sz_sb = ssb.tile([128, 2 * NCH], bf16, tag="szs")
                szs = sz_sb[:].rearrange("p (m ch) -> p m ch", m=2)
                nc.scalar.activation(szs[:, :, :], szv[:, :, :], AF.Sigmoid)

                sn_sb = ssb.tile([128, 2 * NCH], bf16, tag="sns")
                snsv = sn_sb[:].rearrange("p (m ch) -> p m ch", m=2)
                nc.scalar.activation(snsv[:, :, :], spv2[:, :, :], AF.Tanh)

                soz = ssb.tile([128, 2 * NCH], bf16, tag="soz")
                sozv = soz[:].rearrange("p (m ch) -> p m ch", m=2)
                nc.gpsimd.tensor_scalar(sozv[:, :, :], szs[:, :, :],
                                        -1.0, 1.0, Alu.mult, Alu.add)
                szh = ssb.tile([128, 2 * NCH], bf16, tag="szh")
                szhv = szh[:].rearrange("p (m ch) -> p m ch", m=2)
                nc.gpsimd.tensor_mul(szhv[:, :, :], szs[:, :, :],
                                      spv[:, :, :])

                sa = ssb.tile([128, 2 * NCH], bf16, tag="sa")
                sav = sa[:].rearrange("p (m ch) -> p m ch", m=2)
                nc.gpsimd.tensor_mul(sav[:, :, :], snsv[:, :, :], sozv[:, :, :])
                nc.vector.tensor_add(snv[:, :, :], sav[:, :, :], szhv[:, :, :])

                if i % 2 == 1 and i > SW:
                    nc.tensor.matmul(ssum[:, :], sident[:], scur_all[:, :],
                                     start=(i == SW + 1), stop=(i == SSTEPS - 1),
                                     skip_group_check=True)

            # ---------------- P8: means + MLP ----------------
            with tc.tile_pool(name="mlpps", bufs=1, space="PSUM") as mlpps, \
                 tc.tile_pool(name="mlpsb", bufs=2) as mlpsb:
                ssv = ssum[:].rearrange("p (j c ch) -> p j c ch", j=2, c=2)
                r1 = mlpsb.tile([128, 4], f32, tag="r1")
                r1v = r1[:].rearrange("p (j c o) -> p j c o", j=2, o=1)
                nc.vector.tensor_reduce(r1v[:, :, :, :], ssv[:, :, :, :],
                                        axis=AX.X, op=Alu.add)
                hdf = mlpsb.tile([128, 2], f32, tag="hdf")
                hdf_v = hdf[:].rearrange("p (c o) -> p c o", c=2)
                r1cj = r1[:].rearrange("p (j c) -> p c j", j=2)
                nc.vector.tensor_reduce(hdf_v[:, :, :], r1cj[:, :, :],
                                        axis=AX.X, op=Alu.add)
                hdoc = mlpsb.tile([128, 2], bf16, tag="hdoc")
                nc.vector.tensor_scalar_mul(hdoc[:, :], hdf[:, :], 0.5 / NS)
                ps1 = mlpps.tile([128, 1], f32, tag="ps1")
                for kc in range(2):
                    nc.tensor.matmul(ps1[:, :], fc1T_v[:, kc, :],
                                     hdoc[:, kc:kc + 1],
                                     start=(kc == 0), stop=(kc == 1))
                x1 = mlpsb.tile([128, 1], bf16, tag="x1")
                nc.scalar.activation(x1[:, :], ps1[:, :], AF.Sigmoid,
                                     bias=fc1b_sb[:, :])
                ps2 = mlpps.tile([128, 1], f32, tag="ps2")
                nc.tensor.matmul(ps2[:32, :], fc2T[:, :], x1[:, :])
                x2 = mlpsb.tile([32, 1], bf16, tag="x2")
                nc.scalar.activation(x2[:, :], ps2[:32, :], AF.Sigmoid,
                                     bias=fc2b_sb[:, :])
                ps3 = mlpps.tile([128, 1], f32, tag="ps3")
                nc.tensor.matmul(ps3[:1, :], fc3T[:, :], x2[:, :])
                res = mlpsb.tile([1, 1], f32, tag="res")
                nc.scalar.activation(res[:, :], ps3[:1, :], AF.Sigmoid,
                                     bias=fc3b_sb[:, :])
                nc.sync.dma_start(out=out_d[:, :], in_=res[:, :])

    nc.compile()
    return nc


_PROGRAM_CACHE = {}


def _get_program(S, T, n_cores):
    key = (S, T, n_cores)
    if key not in _PROGRAM_CACHE:
        _PROGRAM_CACHE[key] = build_program(S, T, n_cores)
    return _PROGRAM_CACHE[key]


def kernel(**inputs):
    from concourse.bass_utils import run_bass_kernel_spmd

    x = np.ascontiguousarray(np.asarray(inputs["inputs_all"], dtype=np.float32))
    ns, T, _ = x.shape
    S = ns // NCORES
    nc = _get_program(S, T, NCORES)

    weights = {k: np.ascontiguousarray(np.asarray(v, dtype=np.float32))
               for k, v in inputs.items() if k != "inputs_all"}
    in_maps = []
    for c in range(NCORES):
        m = {"x_shard": np.ascontiguousarray(x[c * S:(c + 1) * S])}
        m.update(weights)
        in_maps.append(m)
    res = run_bass_kernel_spmd(nc, in_maps, list(range(NCORES)))
    return np.asarray(res.results[0]["out"], dtype=np.float32)

